# revision 9
# baseline (speedup 1.0000x reference)
"""DynamicLinear (MoE routing) Trainium2 Bass kernel.

Math (per sample b):
    out[b] = sum_k attn[b,k] * (x[b] @ W[k].T + bias[k])
           = sum_k attn[b,k] * (x[b] @ W[k].T) + attn[b] @ bias

Sharding: 8 cores in a 2x4 grid over (batch, out_features).
Each core computes out[b_half, o_quarter] from x[b_half] and
W[:, o_quarter, :] -- no cross-core communication.

The host ships x and W pre-tiled and pre-cast to bf16 in the exact
SBUF layouts the kernel consumes; every load is a plain full-rate
HWDGE DMA.  Matmuls run bf16 x bf16 with fp32 PSUM accumulation.

v2 schedule (from NTFF profile of v1: first MM at 15.3us, 12us of
DMA-starved PE stalls in sweep 0, HAM cold until 25us):
  - 6 dummy warm-up matmuls on a zeroed scratch tile issue right after
    the Tile prologue (~7.3us) so the PE HAM clock-gate is at 8/8 by
    the time real data lands.
  - DMA priority order: sync ring carries attn + expert-0 W in 256 KiB
    granules, then x tiles 8..15, then experts 2..3 (2 MiB each);
    scalar ring carries x tile 0 (split 128K+384K so the first matmul
    can start ~9.5us), x tiles 1..7, then expert 1.  x is consumed at
    ~150 GB/s during sweep 0 -- splitting it across both rings keeps
    delivery ~6us ahead of the PE instead of exactly at parity (the
    cause of v1's mid-sweep stall + HAM re-throttle).
  - 7 rotating PSUM banks for real groups + 1 bank for warm-up.
  - k=3 combine + store run in [128,256] halves to shorten the tail.

Per-core schedule: expert-outer sweeps (k = 0..3); x tiles stay
resident in SBUF after sweep 0.  Per (expert, b_tile): 16 matmul
passes (K=128 contraction, N=512 moving) accumulate in one PSUM bank;
DVE combines acc[t] = sum_k attn[:,k]*(bias[k] + psum_k) with attn as
per-partition scalar; out stores after the last expert.
"""

import numpy as np

_B, _K, _IN, _OUT = 4096, 4, 2048, 2048
_GRID_B, _GRID_O = 2, 4
_BL = _B // _GRID_B      # 2048 batch rows per core
_OL = _OUT // _GRID_O    # 512 out cols per core
_NBT = _BL // 128        # 16 b tiles
_NIT = _IN // 128        # 16 contraction tiles

_CACHE = {}
LAST_RESULTS = None


def _build_program():
    import concourse.bass as bass
    import concourse.tile as tile
    from concourse import bacc, mybir

    f32 = mybir.dt.float32
    bf16 = mybir.dt.bfloat16
    MULT = mybir.AluOpType.mult
    ADD = mybir.AluOpType.add

    nc = bacc.Bacc("TRN2", target_bir_lowering=False, debug=False)
    # host-pretiled layouts: every load is contiguous per partition
    xT = nc.dram_tensor("xT", [_NBT, 128, _NIT, 128], bf16,
                        kind="ExternalInput").ap()
    # attn pre-transposed on host: attn[p, t, k] = attn_orig[t*128+p, k]
    # -- a 16B-granule gather AP here costs ~2048 tiny DMA descriptors
    # that grind the HWDGE ring for ~10us (v2 lesson).
    attn = nc.dram_tensor("attn", [128, _NBT, _K], f32,
                          kind="ExternalInput").ap()
    wT = nc.dram_tensor("wT", [_K, 128, _NIT, _OL], bf16,
                        kind="ExternalInput").ap()
    bias = nc.dram_tensor("bias", [_K, _OL], f32, kind="ExternalInput").ap()
    out = nc.dram_tensor("out", [_BL, _OL], f32, kind="ExternalOutput").ap()

    # expert-0 granule sizes in ii-tiles: small head so the first matmul
    # can start as soon as ~1 ii-tile (128 KiB) of W0 has landed
    _G0 = [1, 1, 2, 2, 2, 2, 2, 2, 2]
    _X0 = [2, 6, 8]   # x-tile-0 piece sizes in ii-tiles
    _NWARM = 6

    with tile.TileContext(nc) as tc:
        with (
            tc.tile_pool(name="w0", bufs=1) as w0p,
            tc.tile_pool(name="wt", bufs=_K - 1) as wtp,
            tc.tile_pool(name="xt", bufs=_NBT - 1) as xtp,
            tc.tile_pool(name="xt0", bufs=1) as xt0p,
            tc.tile_pool(name="singles", bufs=1) as singles,
            tc.tile_pool(name="acc", bufs=_NBT) as accp,
            tc.tile_pool(name="psum", bufs=7, space="PSUM") as psump,
            tc.tile_pool(name="warmps", bufs=1, space="PSUM") as warmpsp,
        ):
            # --- HAM warm-up: zeroed scratch, 6 N=512 matmuls (~2.6us
            # cold) issued before any DMA-dependent work so the PE is at
            # K=8/8 when real data arrives.
            scratch = singles.tile([128, _OL], bf16, name="warm_src")
            nc.vector.memset(scratch, 0)
            warm_ps = warmpsp.tile([128, _OL], f32, name="warm_ps")
            for i in range(_NWARM):
                nc.tensor.matmul(warm_ps, lhsT=scratch[:, 0:128],
                                 rhs=scratch, start=True, stop=True)

            # --- DMA issue order (= per-ring FIFO order) -------------
            # sync ring: W0 granules, xt8..15, W2, W3
            # scalar ring: xt0 pieces, attn, xt1..7, W1, out stores
            # gpsimd: bias broadcast
            w0g = []        # (tile, first_ii, n_ii)
            ii0 = 0
            for g, n in enumerate(_G0):
                t_ = w0p.tile([128, n, _OL], bf16, tag=f"w0g{g}",
                              name=f"w0g{g}")
                nc.sync.dma_start(out=t_, in_=wT[0, :, ii0:ii0 + n])
                w0g.append((t_, ii0, n))
                ii0 += n
            w0_of = {}      # ii -> (tile, offset within tile)
            for t_, first, n in w0g:
                for j in range(n):
                    w0_of[first + j] = (t_, j)

            xt0 = []
            ii0 = 0
            for p, n in enumerate(_X0):
                t_ = xt0p.tile([128, n, 128], bf16, tag=f"xt0p{p}",
                               name=f"xt0p{p}")
                nc.scalar.dma_start(out=t_, in_=xT[0, :, ii0:ii0 + n])
                xt0.append((t_, ii0, n))
                ii0 += n
            xt0_of = {}
            for t_, first, n in xt0:
                for j in range(n):
                    xt0_of[first + j] = (t_, j)

            attn_sb = singles.tile([128, _NBT, _K], f32)
            nc.scalar.dma_start(out=attn_sb, in_=attn)

            # bias replicated across all 128 partitions (SWDGE, small)
            bias_rep = singles.tile([128, _K, _OL], f32)
            nc.gpsimd.dma_start(
                out=bias_rep,
                in_=bass.AP(
                    tensor=bias.tensor,
                    offset=bias.offset,
                    ap=[[0, 128], bias.ap[0], bias.ap[1]],
                ),
            )

            xts = {}
            for t in range(1, 8):
                xts[t] = xtp.tile([128, _NIT, 128], bf16, tag="xt",
                                  name=f"xt{t}")
                nc.scalar.dma_start(out=xts[t], in_=xT[t])
            for t in range(8, _NBT):
                xts[t] = xtp.tile([128, _NIT, 128], bf16, tag="xt",
                                  name=f"xt{t}")
                nc.sync.dma_start(out=xts[t], in_=xT[t])

            wt = {}
            wt[1] = wtp.tile([128, _NIT, _OL], bf16, tag="wt", name="wt1")
            nc.scalar.dma_start(out=wt[1], in_=wT[1])
            for k in (2, 3):
                wt[k] = wtp.tile([128, _NIT, _OL], bf16, tag="wt",
                                 name=f"wt{k}")
                nc.sync.dma_start(out=wt[k], in_=wT[k])

            def rhs_of(k, ii):
                if k == 0:
                    t_, j = w0_of[ii]
                    return t_[:, j, :]
                return wt[k][:, ii, :]

            def lhsT_of(t, ii):
                if t == 0:
                    t_, j = xt0_of[ii]
                    return t_[:, j, :]
                return xts[t][:, ii, :]

            # --- main sweeps -----------------------------------------
            acc = [None] * _NBT
            for k in range(_K):
                for t in range(_NBT):
                    a_sc = attn_sb[:, t, :]
                    ps = psump.tile([128, _OL], f32, tag="ps",
                                    name=f"ps{k}_{t}")
                    for ii in range(_NIT):
                        nc.tensor.matmul(
                            ps,
                            lhsT=lhsT_of(t, ii),
                            rhs=rhs_of(k, ii),
                            start=(ii == 0), stop=(ii == _NIT - 1),
                        )
                    if k == 0:
                        # init acc with the full bias combination (DVE)
                        at = accp.tile([128, _OL], f32, tag="acc",
                                       name=f"acc{t}")
                        acc[t] = at
                        nc.vector.tensor_scalar(
                            out=at, in0=bias_rep[:, 0, :],
                            scalar1=a_sc[:, 0:1], scalar2=None, op0=MULT,
                        )
                        for kk in range(1, _K):
                            nc.vector.scalar_tensor_tensor(
                                out=at, in0=bias_rep[:, kk, :],
                                scalar=a_sc[:, kk:kk + 1], in1=at,
                                op0=MULT, op1=ADD,
                            )
                    if k < _K - 1:
                        nc.vector.scalar_tensor_tensor(
                            out=acc[t], in0=ps, scalar=a_sc[:, k:k + 1],
                            in1=acc[t], op0=MULT, op1=ADD,
                        )
                    else:
                        # final sweep: combine + store in halves so the
                        # kernel tail is one [128,256] op + store, not
                        # a full-tile chain.
                        for h in range(2):
                            sl = slice(h * 256, (h + 1) * 256)
                            nc.vector.scalar_tensor_tensor(
                                out=acc[t][:, sl], in0=ps[:, sl],
                                scalar=a_sc[:, k:k + 1],
                                in1=acc[t][:, sl], op0=MULT, op1=ADD,
                            )
                            nc.scalar.dma_start(
                                out=out[t * 128:(t + 1) * 128, sl],
                                in_=acc[t][:, sl],
                            )

    nc.compile()
    return nc


def _get_program():
    if "nc" not in _CACHE:
        _CACHE["nc"] = _build_program()
    return _CACHE["nc"]


def _ensure_axon_hooks_importable():
    """bass_utils' trace branch imports antenv.axon_hooks, which the
    trimmed agent image may lack; stub it (hook=None) so a stray
    BASS_TRACE=1 degrades to an untraced run instead of crashing."""
    import sys
    import types

    try:
        import antenv.axon_hooks  # noqa: F401
        return
    except ImportError:
        pass
    mod = types.ModuleType("antenv.axon_hooks")
    mod._hook = None
    mod.get_axon_ntff_profile_hook = lambda: mod._hook

    def _set(h):
        mod._hook = h

    mod.set_axon_ntff_profile_hook = _set
    sys.modules["antenv.axon_hooks"] = mod
    try:
        import antenv
        antenv.axon_hooks = mod
    except ImportError:
        pass


def kernel(**inputs):
    global LAST_RESULTS
    from concourse.bass_utils import run_bass_kernel_spmd

    _ensure_axon_hooks_importable()

    x = np.ascontiguousarray(inputs["x"], dtype=np.float32)
    attn = np.ascontiguousarray(inputs["softmax_attention"], dtype=np.float32)
    w = np.ascontiguousarray(inputs["weight"], dtype=np.float32)
    b = np.ascontiguousarray(inputs["bias"], dtype=np.float32)

    nc = _get_program()
    in_maps = []
    for c in range(8):
        gb, go = divmod(c, _GRID_O)
        x_sl = x[gb * _BL:(gb + 1) * _BL]
        w_sl = w[:, go * _OL:(go + 1) * _OL, :]
        # tile-contiguous device layouts (see _build_program):
        # xT[t, i_in, ii, b_in] = x[t*128 + b_in, ii*128 + i_in]
        # wT[k, i_in, ii, o]    = W[k, o, ii*128 + i_in]
        import ml_dtypes
        xT = np.ascontiguousarray(
            x_sl.T.reshape(_NIT, 128, _NBT, 128).transpose(2, 1, 0, 3)
        ).astype(ml_dtypes.bfloat16)
        wTa = np.ascontiguousarray(
            w_sl.transpose(0, 2, 1)
            .reshape(_K, _NIT, 128, _OL).transpose(0, 2, 1, 3)
        ).astype(ml_dtypes.bfloat16)
        # attn pre-transposed: attnT[p, t, k] = attn[t*128 + p, k]
        attnT = np.ascontiguousarray(
            attn[gb * _BL:(gb + 1) * _BL]
            .reshape(_NBT, 128, _K).transpose(1, 0, 2)
        )
        in_maps.append({
            "xT": xT,
            "attn": attnT,
            "wT": wTa,
            "bias": np.ascontiguousarray(b[:, go * _OL:(go + 1) * _OL]),
        })

    res = run_bass_kernel_spmd(nc, in_maps, list(range(8)))
    LAST_RESULTS = res

    full = np.empty((_B, _OUT), dtype=np.float32)
    for c in range(8):
        gb, go = divmod(c, _GRID_O)
        full[gb * _BL:(gb + 1) * _BL, go * _OL:(go + 1) * _OL] = \
            res.results[c]["out"]
    return full


# revision 10
# speedup vs baseline: 1.0236x; 1.0236x over previous
"""DynamicLinear (MoE routing) Trainium2 Bass kernel.

Math (per sample b):
    out[b] = sum_k attn[b,k] * (x[b] @ W[k].T + bias[k])
           = sum_k attn[b,k] * (x[b] @ W[k].T) + attn[b] @ bias

Sharding: 8 cores in a 2x4 grid over (batch, out_features).
Each core computes out[b_half, o_quarter] from x[b_half] and
W[:, o_quarter, :] -- no cross-core communication.

The host ships x and W pre-tiled and pre-cast to bf16 in the exact
SBUF layouts the kernel consumes; every load is a plain full-rate
HWDGE DMA.  Matmuls run bf16 x bf16 with fp32 PSUM accumulation.

v2 schedule (from NTFF profile of v1: first MM at 15.3us, 12us of
DMA-starved PE stalls in sweep 0, HAM cold until 25us):
  - 6 dummy warm-up matmuls on a zeroed scratch tile issue right after
    the Tile prologue (~7.3us) so the PE HAM clock-gate is at 8/8 by
    the time real data lands.
  - DMA priority order: sync ring carries attn + expert-0 W in 256 KiB
    granules, then x tiles 8..15, then experts 2..3 (2 MiB each);
    scalar ring carries x tile 0 (split 128K+384K so the first matmul
    can start ~9.5us), x tiles 1..7, then expert 1.  x is consumed at
    ~150 GB/s during sweep 0 -- splitting it across both rings keeps
    delivery ~6us ahead of the PE instead of exactly at parity (the
    cause of v1's mid-sweep stall + HAM re-throttle).
  - 7 rotating PSUM banks for real groups + 1 bank for warm-up.
  - k=3 combine + store run in [128,256] halves to shorten the tail.

Per-core schedule: expert-outer sweeps (k = 0..3); x tiles stay
resident in SBUF after sweep 0.  Per (expert, b_tile): 16 matmul
passes (K=128 contraction, N=512 moving) accumulate in one PSUM bank;
DVE combines acc[t] = sum_k attn[:,k]*(bias[k] + psum_k) with attn as
per-partition scalar; out stores after the last expert.
"""

import numpy as np

_B, _K, _IN, _OUT = 4096, 4, 2048, 2048
_GRID_B, _GRID_O = 2, 4
_BL = _B // _GRID_B      # 2048 batch rows per core
_OL = _OUT // _GRID_O    # 512 out cols per core
_NBT = _BL // 128        # 16 b tiles
_NIT = _IN // 128        # 16 contraction tiles

_CACHE = {}
LAST_RESULTS = None


def _build_program():
    import concourse.bass as bass
    import concourse.tile as tile
    from concourse import bacc, mybir

    f32 = mybir.dt.float32
    bf16 = mybir.dt.bfloat16
    MULT = mybir.AluOpType.mult
    ADD = mybir.AluOpType.add

    nc = bacc.Bacc("TRN2", target_bir_lowering=False, debug=False)
    # host-pretiled layouts: every load is contiguous per partition
    xT = nc.dram_tensor("xT", [_NBT, 128, _NIT, 128], bf16,
                        kind="ExternalInput").ap()
    # attn pre-transposed on host: attn[p, t, k] = attn_orig[t*128+p, k]
    # -- a 16B-granule gather AP here costs ~2048 tiny DMA descriptors
    # that grind the HWDGE ring for ~10us (v2 lesson).
    attn = nc.dram_tensor("attn", [128, _NBT, _K], f32,
                          kind="ExternalInput").ap()
    wT = nc.dram_tensor("wT", [_K, 128, _NIT, _OL], bf16,
                        kind="ExternalInput").ap()
    bias = nc.dram_tensor("bias", [_K, _OL], f32, kind="ExternalInput").ap()
    out = nc.dram_tensor("out", [_BL, _OL], f32, kind="ExternalOutput").ap()

    # expert-0 granule sizes in ii-tiles: small head so the first matmul
    # can start as soon as ~1 ii-tile (128 KiB) of W0 has landed
    _G0 = [1, 1, 2, 2, 2, 2, 2, 2, 2]
    _X0 = [2, 6, 8]   # x-tile-0 piece sizes in ii-tiles
    _NWARM = 6

    with tile.TileContext(nc) as tc:
        with (
            tc.tile_pool(name="w0", bufs=1) as w0p,
            tc.tile_pool(name="wt", bufs=_K - 1) as wtp,
            tc.tile_pool(name="xt", bufs=_NBT - 1) as xtp,
            tc.tile_pool(name="xt0", bufs=1) as xt0p,
            tc.tile_pool(name="singles", bufs=1) as singles,
            tc.tile_pool(name="acc", bufs=_NBT) as accp,
            tc.tile_pool(name="psum", bufs=7, space="PSUM") as psump,
            tc.tile_pool(name="warmps", bufs=1, space="PSUM") as warmpsp,
        ):
            # --- HAM warm-up: zeroed scratch, 6 N=512 matmuls (~2.6us
            # cold) issued before any DMA-dependent work so the PE is at
            # K=8/8 when real data arrives.
            scratch = singles.tile([128, _OL], bf16, name="warm_src")
            nc.vector.memset(scratch, 0)
            warm_ps = warmpsp.tile([128, _OL], f32, name="warm_ps")
            for i in range(_NWARM):
                nc.tensor.matmul(warm_ps, lhsT=scratch[:, 0:128],
                                 rhs=scratch, start=True, stop=True)

            # --- DMA issue order (= per-ring FIFO order) -------------
            # The scalar (Act) HWDGE ring starts draining ~2.7us before
            # the sync (SP) ring (8.7us vs 11.5us, every trace), so the
            # head-critical pieces go on scalar:
            # scalar: w0g0, xt0a, w0g1, w0g2, xt0b, xt0c, attn,
            #         xt1..7, W1, out stores
            # sync:   w0g3..w0g8, xt8..15, W2, W3
            # gpsimd: bias broadcast
            _N_W0_SCALAR = 3
            w0g = []        # (tile, first_ii, n_ii)

            def load_w0(g, ii0, n, eng):
                t_ = w0p.tile([128, n, _OL], bf16, tag=f"w0g{g}",
                              name=f"w0g{g}")
                eng.dma_start(out=t_, in_=wT[0, :, ii0:ii0 + n])
                w0g.append((t_, ii0, n))

            xt0 = []

            def load_x0(p, ii0, n):
                t_ = xt0p.tile([128, n, 128], bf16, tag=f"xt0p{p}",
                               name=f"xt0p{p}")
                nc.scalar.dma_start(out=t_, in_=xT[0, :, ii0:ii0 + n])
                xt0.append((t_, ii0, n))

            # interleaved head on the fast ring
            load_w0(0, 0, _G0[0], nc.scalar)
            load_x0(0, 0, _X0[0])
            load_w0(1, _G0[0], _G0[1], nc.scalar)
            load_w0(2, _G0[0] + _G0[1], _G0[2], nc.scalar)
            load_x0(1, _X0[0], _X0[1])
            load_x0(2, _X0[0] + _X0[1], _X0[2])

            ii0 = sum(_G0[:_N_W0_SCALAR])
            for g in range(_N_W0_SCALAR, len(_G0)):
                load_w0(g, ii0, _G0[g], nc.sync)
                ii0 += _G0[g]

            w0_of = {}      # ii -> (tile, offset within tile)
            for t_, first, n in w0g:
                for j in range(n):
                    w0_of[first + j] = (t_, j)
            xt0_of = {}
            for t_, first, n in xt0:
                for j in range(n):
                    xt0_of[first + j] = (t_, j)

            attn_sb = singles.tile([128, _NBT, _K], f32)
            nc.scalar.dma_start(out=attn_sb, in_=attn)

            # bias replicated across all 128 partitions (SWDGE, small)
            bias_rep = singles.tile([128, _K, _OL], f32)
            nc.gpsimd.dma_start(
                out=bias_rep,
                in_=bass.AP(
                    tensor=bias.tensor,
                    offset=bias.offset,
                    ap=[[0, 128], bias.ap[0], bias.ap[1]],
                ),
            )

            xts = {}
            for t in range(1, 8):
                xts[t] = xtp.tile([128, _NIT, 128], bf16, tag="xt",
                                  name=f"xt{t}")
                nc.scalar.dma_start(out=xts[t], in_=xT[t])
            for t in range(8, _NBT):
                xts[t] = xtp.tile([128, _NIT, 128], bf16, tag="xt",
                                  name=f"xt{t}")
                nc.sync.dma_start(out=xts[t], in_=xT[t])

            wt = {}
            wt[1] = wtp.tile([128, _NIT, _OL], bf16, tag="wt", name="wt1")
            nc.scalar.dma_start(out=wt[1], in_=wT[1])
            for k in (2, 3):
                wt[k] = wtp.tile([128, _NIT, _OL], bf16, tag="wt",
                                 name=f"wt{k}")
                nc.sync.dma_start(out=wt[k], in_=wT[k])

            def rhs_of(k, ii):
                if k == 0:
                    t_, j = w0_of[ii]
                    return t_[:, j, :]
                return wt[k][:, ii, :]

            def lhsT_of(t, ii):
                if t == 0:
                    t_, j = xt0_of[ii]
                    return t_[:, j, :]
                return xts[t][:, ii, :]

            # --- main sweeps -----------------------------------------
            acc = [None] * _NBT
            for k in range(_K):
                for t in range(_NBT):
                    a_sc = attn_sb[:, t, :]
                    ps = psump.tile([128, _OL], f32, tag="ps",
                                    name=f"ps{k}_{t}")
                    for ii in range(_NIT):
                        nc.tensor.matmul(
                            ps,
                            lhsT=lhsT_of(t, ii),
                            rhs=rhs_of(k, ii),
                            start=(ii == 0), stop=(ii == _NIT - 1),
                        )
                    if k == 0:
                        # init acc with the full bias combination (DVE)
                        at = accp.tile([128, _OL], f32, tag="acc",
                                       name=f"acc{t}")
                        acc[t] = at
                        nc.vector.tensor_scalar(
                            out=at, in0=bias_rep[:, 0, :],
                            scalar1=a_sc[:, 0:1], scalar2=None, op0=MULT,
                        )
                        for kk in range(1, _K):
                            nc.vector.scalar_tensor_tensor(
                                out=at, in0=bias_rep[:, kk, :],
                                scalar=a_sc[:, kk:kk + 1], in1=at,
                                op0=MULT, op1=ADD,
                            )
                    if k < _K - 1:
                        nc.vector.scalar_tensor_tensor(
                            out=acc[t], in0=ps, scalar=a_sc[:, k:k + 1],
                            in1=acc[t], op0=MULT, op1=ADD,
                        )
                    else:
                        # final sweep: combine + store in halves so the
                        # kernel tail is one [128,256] op + store, not
                        # a full-tile chain.
                        for h in range(2):
                            sl = slice(h * 256, (h + 1) * 256)
                            nc.vector.scalar_tensor_tensor(
                                out=acc[t][:, sl], in0=ps[:, sl],
                                scalar=a_sc[:, k:k + 1],
                                in1=acc[t][:, sl], op0=MULT, op1=ADD,
                            )
                            nc.scalar.dma_start(
                                out=out[t * 128:(t + 1) * 128, sl],
                                in_=acc[t][:, sl],
                            )

    nc.compile()
    return nc


def _get_program():
    if "nc" not in _CACHE:
        _CACHE["nc"] = _build_program()
    return _CACHE["nc"]


def _ensure_axon_hooks_importable():
    """bass_utils' trace branch imports antenv.axon_hooks, which the
    trimmed agent image may lack; stub it (hook=None) so a stray
    BASS_TRACE=1 degrades to an untraced run instead of crashing."""
    import sys
    import types

    try:
        import antenv.axon_hooks  # noqa: F401
        return
    except ImportError:
        pass
    mod = types.ModuleType("antenv.axon_hooks")
    mod._hook = None
    mod.get_axon_ntff_profile_hook = lambda: mod._hook

    def _set(h):
        mod._hook = h

    mod.set_axon_ntff_profile_hook = _set
    sys.modules["antenv.axon_hooks"] = mod
    try:
        import antenv
        antenv.axon_hooks = mod
    except ImportError:
        pass


def kernel(**inputs):
    global LAST_RESULTS
    from concourse.bass_utils import run_bass_kernel_spmd

    _ensure_axon_hooks_importable()

    x = np.ascontiguousarray(inputs["x"], dtype=np.float32)
    attn = np.ascontiguousarray(inputs["softmax_attention"], dtype=np.float32)
    w = np.ascontiguousarray(inputs["weight"], dtype=np.float32)
    b = np.ascontiguousarray(inputs["bias"], dtype=np.float32)

    nc = _get_program()
    in_maps = []
    for c in range(8):
        gb, go = divmod(c, _GRID_O)
        x_sl = x[gb * _BL:(gb + 1) * _BL]
        w_sl = w[:, go * _OL:(go + 1) * _OL, :]
        # tile-contiguous device layouts (see _build_program):
        # xT[t, i_in, ii, b_in] = x[t*128 + b_in, ii*128 + i_in]
        # wT[k, i_in, ii, o]    = W[k, o, ii*128 + i_in]
        import ml_dtypes
        xT = np.ascontiguousarray(
            x_sl.T.reshape(_NIT, 128, _NBT, 128).transpose(2, 1, 0, 3)
        ).astype(ml_dtypes.bfloat16)
        wTa = np.ascontiguousarray(
            w_sl.transpose(0, 2, 1)
            .reshape(_K, _NIT, 128, _OL).transpose(0, 2, 1, 3)
        ).astype(ml_dtypes.bfloat16)
        # attn pre-transposed: attnT[p, t, k] = attn[t*128 + p, k]
        attnT = np.ascontiguousarray(
            attn[gb * _BL:(gb + 1) * _BL]
            .reshape(_NBT, 128, _K).transpose(1, 0, 2)
        )
        in_maps.append({
            "xT": xT,
            "attn": attnT,
            "wT": wTa,
            "bias": np.ascontiguousarray(b[:, go * _OL:(go + 1) * _OL]),
        })

    res = run_bass_kernel_spmd(nc, in_maps, list(range(8)))
    LAST_RESULTS = res

    full = np.empty((_B, _OUT), dtype=np.float32)
    for c in range(8):
        gb, go = divmod(c, _GRID_O)
        full[gb * _BL:(gb + 1) * _BL, go * _OL:(go + 1) * _OL] = \
            res.results[c]["out"]
    return full
